# revision 37
# baseline (speedup 1.0000x reference)
"""Inverse DTCWT (biort bandpass) level-1 reconstruction as a Bass/Tile kernel.

Math: the reference is
    y = (A0 @ Yl + A1 @ lh) @ A0^T + (A0 @ hl) @ A1^T + (A2 @ hh) @ A2^T
where A* are 256x256 banded matrices (1D taps + symmetric padding folded in)
and lh/hl/hh are the c2q quad-interleaves of subband pairs (0,5)/(2,3)/(1,4).

Row r of a c2q image comes from `top` (r even) or `bot` (r odd), each a
128x256 column-interleaved image built with 3 tensor-tensor ops per pair:
    top[:, 0::2] = w1r + w2r ; top[:, 1::2] = w1i + w2i
    bot[:, 0::2] = w1i - w2i ; bot[:, 1::2] = w2r - w1r
The row interleave never materializes: contraction over rows splits into
even/odd with host-precomputed matrices Re = A^T[0::2]/sqrt2, Ro = A^T[1::2]/sqrt2.

Stage A (col filters) runs with the image tiles stationary producing
transposed intermediates Z[c, h] in PSUM; stage B (row filters) consumes Z
slices as stationary against A^T and accumulates all three paths into one
PSUM bank in natural orientation. No transposes anywhere.

Everything runs in fp16: halves the HBM traffic vs fp32 and runs the PE at
1 cycle/row at ANY moving width (fp32r needs >=256-wide moving for full
rate). That unlocks banded windows: A^T is banded (13/19/13 taps), so the
Yl stage-A matmuls and all 12 stage-B matmuls stream only the ~134-137
nonzero columns of each 256-wide A^T chunk. PSUM start=True marks the whole
2KB bank pending-zero and each byte's first writer overwrites, so one start
per bank is enough even with partial windows.

top/bot are linear combos of the input subbands, so the host computes them
during packing (c2q is a bijective repack: same DMA bytes, zero on-device
element-wise work). They are packed parity-stacked (partitions 0-63 = top
half-slab, 64-127 = bot half-slab) so one matmul contracts both parities of
a half-height slab against the stacked [Re; Ro] matrix, whose nonzero
window is only 128+m columns. Bank 0 uses the windowed form; banks 1-2
stream the full 256 columns on purpose — the PSUM->SBUF copy engines
(vector: z0+out, scalar: z1+z2, ~2.1us/image between them) pin the steady
period anyway, and a longer stage A keeps them below the saturation point
where copy backpressure quantizes into PE stalls.

The per-image schedule is software-pipelined: stage B of image i-1 issues
after stage A of image i, and the copies drain during the opposite stage so
the PE never waits on one. Inputs stream on the sync DMA queue (consts on
scalar, yl/out on gpsimd), group 0 split per-image; a few dummy matmuls
ramp the PE clock while the first transfers land.

Sharding: pure data parallel, batch dim (8) across 8 cores.
"""
import sys

if "/opt/trn_rl_repo" not in sys.path:
    sys.path.insert(0, "/opt/trn_rl_repo")

import numpy as np

_C, _H = 64, 256  # channels per core, image size
_NCORES = 8
_G = 4  # images (channels) per group


def _band_matrix(h, N):
    """A @ x == colfilter(x, h) with symmetric padding, in float64."""
    h = np.asarray(h, dtype=np.float64)
    L = h.shape[0]
    m = L // 2
    A = np.zeros((N, N), dtype=np.float64)
    for i in range(N):
        for k in range(L):
            s = i + k - m
            if s < 0:
                s = -1 - s
            elif s >= N:
                s = 2 * N - 1 - s
            A[i, s] += h[L - 1 - k]
    return A


def build_consts(g0o, g1o, g2o):
    """Host-side constant tensors handed to every core."""
    A0 = _band_matrix(g0o, _H).T  # stored transposed: [r, h]
    A1 = _band_matrix(g1o, _H).T
    A2 = _band_matrix(g2o, _H).T
    s2 = np.sqrt(2.0)

    def tile2(AT):  # [256, 256] -> [128, 2, 256] with [p, kr, h] = AT[128*kr+p, h]
        return np.ascontiguousarray(
            AT.reshape(2, 128, 256).transpose(1, 0, 2)
        ).astype(np.float16)

    a0t, a1t, a2t = tile2(A0), tile2(A1), tile2(A2)
    # rmats[q, e/o]: per-pair col-filter matrices; pair q uses bands (q, 5-q):
    #   q=0 (lh)   -> col filter A1 ; q=1 (hh) -> A2 ; q=2 (hl) -> A0
    rmats = np.stack(
        [
            np.stack([A1[0::2] / s2, A1[1::2] / s2]),
            np.stack([A2[0::2] / s2, A2[1::2] / s2]),
            np.stack([A0[0::2] / s2, A0[1::2] / s2]),
        ]
    )  # [3, e/o, 128, 256]
    # parity-stacked layout [t, q, h, hout]: partition t<64 holds the even
    # (top) rows 64h+t, t>=64 the odd (bot) rows 64h+t-64. One matmul then
    # contracts a half-height slab of BOTH parities, and the nonzero hout
    # window shrinks to 128+m — windowed matmuls stream ~2x faster than
    # 256-wide ones.
    rmp = np.empty((128, 3, 2, 256), dtype=np.float16)
    rmp[:64] = rmats[:, 0].reshape(3, 2, 64, 256).transpose(2, 0, 1, 3)
    rmp[64:] = rmats[:, 1].reshape(3, 2, 64, 256).transpose(2, 0, 1, 3)
    return {"a0t": a0t, "a1t": a1t, "a2t": a2t, "rmats": rmp}


# half-tap margins of the three band matrices (13/19/13 taps)
_M0, _M1, _M2 = 6, 9, 6


def build_nc(n_images):
    import concourse.bacc as bacc
    import concourse.mybir as mybir
    from concourse.tile import TileContext

    f32 = mybir.dt.float32
    f16 = mybir.dt.float16
    nc = bacc.Bacc(None, target_bir_lowering=False, debug=False)

    n_groups = n_images // _G
    yl_d = nc.declare_dram_parameter(
        "ylp", [n_groups, 128, _G, 2, 256], f16, isOutput=False
    )
    tb_d = nc.declare_dram_parameter(
        "tbp", [n_groups, 128, _G, 3, 2, 128, 2], f16, isOutput=False
    )
    a0t_d = nc.declare_dram_parameter("a0t", [128, 2, 256], f16, isOutput=False)
    a1t_d = nc.declare_dram_parameter("a1t", [128, 2, 256], f16, isOutput=False)
    a2t_d = nc.declare_dram_parameter("a2t", [128, 2, 256], f16, isOutput=False)
    rm_d = nc.declare_dram_parameter("rmats", [128, 3, 2, 256], f16, isOutput=False)
    out_d = nc.declare_dram_parameter(
        "out", [n_groups, 128, _G, 2, 256], f16, isOutput=True
    )
    assert n_groups * _G == n_images

    with TileContext(nc) as tc:
        with (
            tc.tile_pool(name="consts", bufs=1) as cpool,
            tc.tile_pool(name="io", bufs=2) as io_pool,
            tc.tile_pool(name="tb", bufs=2) as tb_pool,
            tc.tile_pool(name="zsb", bufs=2) as z_pool,
            tc.tile_pool(name="ps", bufs=2, space="PSUM") as ps_pool,
        ):
            a0t = cpool.tile([128, 2, 256], f16)
            a1t = cpool.tile([128, 2, 256], f16)
            a2t = cpool.tile([128, 2, 256], f16)
            amats = (a0t, a1t, a2t)
            ms = (_M0, _M1, _M2)
            rm = cpool.tile([128, 3, 2, 256], f16)

            # PE p-state warmup: dummy matmuls on zeroed scratch while the
            # first DMAs land, so real work starts at full clock
            scratch = cpool.tile([128, 512], f16)
            nc.vector.memset(scratch[:], 0.0)
            wp = ps_pool.tile([128, 2, 256], f32, tag="yp")
            for _ in range(8):
                nc.tensor.matmul(
                    wp[:, :, :], scratch[:, 0:128], scratch[:],
                    start=True, stop=True,
                )

            # consts on the scalar DMA queue, data on sync: parallel transfers
            nc.scalar.dma_start(rm[:], rm_d[:])
            nc.scalar.dma_start(a0t[:], a0t_d[:])
            nc.scalar.dma_start(a1t[:], a1t_d[:])
            nc.scalar.dma_start(a2t[:], a2t_d[:])

            # software pipeline state: stage B of image idx-1 runs behind
            # stage A of image idx
            prev = None  # (z_s, out_sb, i, g, idx) of the previous image

            def stage_b(z_s, out_sb, i, g, idx):
                # p outer / r inner: consumes z banks in the order their
                # copies complete (bank 0 first)
                yp = ps_pool.tile([128, 2, 256], f32, tag="yp")
                nmm = 0
                for p in range(3):
                    m = ms[p]
                    for r in range(2):
                        rs = slice(128 * r, 128 * r + 128)
                        for k in range(2):
                            w0, w1 = (0, 128 + m) if k == 0 else (128 - m, 256)
                            nc.tensor.matmul(
                                yp[:, r, w0:w1],
                                z_s[:, p, k, rs],
                                amats[p][:, k, w0:w1],
                                start=(nmm == 0),
                                stop=(nmm == 11),
                            )
                            nmm += 1
                # out copy on vector; DMA issues from the gpsimd queue
                nc.vector.tensor_copy(out_sb[:, i, :, :], yp[:])
                nc.gpsimd.dma_start(out_d[g][:, i], out_sb[:, i])

            for idx in range(n_images):
                g, i = divmod(idx, _G)
                if i == 0:
                    tb = tb_pool.tile(
                        [128, _G, 3, 2, 128, 2], f16, tag="tb", bufs=3
                    )
                    yl = io_pool.tile([128, _G, 2, 256], f16, tag="yl")
                    out_sb = io_pool.tile([128, _G, 2, 256], f16, tag="out_sb")
                    # group 0 split per-image so early images land first
                    if g == 0:
                        nc.sync.dma_start(tb[:, 0:1], tb_d[g][:, 0:1])
                        nc.sync.dma_start(yl[:, 0:1], yl_d[g][:, 0:1])
                        nc.sync.dma_start(tb[:, 1:2], tb_d[g][:, 1:2])
                        nc.sync.dma_start(yl[:, 1:_G], yl_d[g][:, 1:_G])
                        nc.sync.dma_start(tb[:, 2:_G], tb_d[g][:, 2:_G])
                    else:
                        nc.sync.dma_start(tb[:], tb_d[g])
                        nc.gpsimd.dma_start(yl[:], yl_d[g])

                # ---- stage A: Z[c, h] = col-filtered, transposed ----
                # z banks: 0 = y1 path (lh via rm[0] + Yl via a0t, row filter A0)
                #          1 = hl path (pair q=2, col rm[2], row filter A1)
                #          2 = hh path (pair q=1, col rm[1], row filter A2)
                z = ps_pool.tile([128, 3, 2, 256], f32, tag="z")

                # per-q col-filter margins: q=0 (lh) uses A1, q=1 (hh) A2,
                # q=2 (hl) A0
                colm = (_M1, _M2, _M0)

                def subs(bank, q, last_stop, windowed, i=i, z=z):
                    mq = colm[q]
                    for cc in range(2):
                        js = slice(64 * cc, 64 * cc + 64)
                        for h in range(2):
                            if windowed:
                                w0, w1 = (
                                    (0, 128 + mq) if h == 0 else (128 - mq, 256)
                                )
                            else:
                                w0, w1 = 0, 256
                            nc.tensor.matmul(
                                z[:, bank, cc, w0:w1],
                                tb[:, i, q, h, js, :],
                                rm[:, q, h, w0:w1],
                                start=(cc == 0 and h == 0),
                                stop=(last_stop and cc == 1 and h == 1),
                            )

                def bank0(i=i, z=z):
                    subs(0, 0, last_stop=False, windowed=True)
                    for cc in range(2):
                        ws = slice(128 * cc, 128 * cc + 128)
                        for k in range(2):
                            w0, w1 = (
                                (0, 128 + _M0) if k == 0 else (128 - _M0, 256)
                            )
                            nc.tensor.matmul(
                                z[:, 0, cc, w0:w1], yl[:, i, k, ws],
                                a0t[:, k, w0:w1],
                                start=False, stop=(cc == 1 and k == 1),
                            )

                def bank12(i=i, z=z):
                    subs(1, 2, last_stop=True, windowed=True)
                    subs(2, 1, last_stop=True, windowed=True)

                # bank 0 first so its copy (stage B's first dependency)
                # starts earliest; in the first group the yl transfer is
                # still in flight, so run the tb-only banks first instead
                if idx < _G:
                    bank12()
                    bank0()
                else:
                    bank0()
                    bank12()

                # PSUM -> SBUF fp16 in bank-completion order, balanced
                # across vector + scalar (bank 1 split between them)
                z_s = z_pool.tile([128, 3, 2, 256], f16, tag="zs")
                nc.vector.tensor_copy(z_s[:, 0], z[:, 0])
                nc.scalar.copy(z_s[:, 1:3], z[:, 1:3])

                # ---- stage B for the previous image ----
                if prev is not None:
                    stage_b(*prev)
                prev = (z_s, out_sb, i, g, idx)

            stage_b(*prev)
    nc.compile()
    return nc


_NC_CACHE = {}


def _get_nc(n_images):
    if n_images not in _NC_CACHE:
        _NC_CACHE[n_images] = build_nc(n_images)
    return _NC_CACHE[n_images]


def pack_inputs(Yl_k, Yhr_k, Yhi_k):
    """Per-core repack into group-major fp16 layouts with long contiguous rows.

    The c2q column-interleaved top/bot half-images are linear combos of the
    subbands; build them here so the device does matmuls only.
    tbp[g, h, i, q, t/b, w, ri]                       -> 12KB/partition/group
    ylp[g, p, i, k, w] = Yl[4g+i, 128k+p, w]          ->  4KB/partition/group
    """
    ng = _C // _G
    R = Yhr_k.reshape(ng, _G, 6, 128, 128)
    I = Yhi_k.reshape(ng, _G, 6, 128, 128)
    # parity-stacked: partition p<64 holds top rows 64h+p, p>=64 bot rows
    # 64h+p-64, matching the stacked rmats layout
    tbp = np.empty((ng, 128, _G, 3, 2, 128, 2), dtype=np.float16)
    for q in range(3):
        w1r, w2r = R[:, :, q], R[:, :, 5 - q]
        w1i, w2i = I[:, :, q], I[:, :, 5 - q]
        for comp, dst in (
            (w1r + w2r, tbp[:, 0:64, :, q, :, :, 0]),
            (w1i + w2i, tbp[:, 0:64, :, q, :, :, 1]),
            (w1i - w2i, tbp[:, 64:128, :, q, :, :, 0]),
            (w2r - w1r, tbp[:, 64:128, :, q, :, :, 1]),
        ):
            # [ng, G, 128h'', w] -> split h''=64h+p -> [ng, p, G, h, w]
            dst[:] = comp.reshape(ng, _G, 2, 64, 128).transpose(0, 3, 1, 2, 4)
    ylp = np.ascontiguousarray(
        Yl_k.reshape(ng, _G, 2, 128, 256).transpose(0, 3, 1, 2, 4)
    ).astype(np.float16)
    return tbp, ylp


def unpack_output(outp):
    """outp (ng, 128, G, 2, 256): [g, p, i, k, w] = y[Gg+i, 128k+p, w]."""
    return np.ascontiguousarray(
        outp.transpose(0, 2, 3, 1, 4).reshape(outp.shape[0] * _G, 256, 256)
    )


def kernel(Yl, Yhr, Yhi, g0o, g1o, g2o):
    from concourse.bass_utils import run_bass_kernel_spmd

    # banded windows in build_nc assume 13/19/13-tap filters
    assert len(g0o) == 13 and len(g1o) == 19 and len(g2o) == 13

    Yl = np.asarray(Yl, dtype=np.float32)
    Yhr = np.asarray(Yhr, dtype=np.float32)
    Yhi = np.asarray(Yhi, dtype=np.float32)
    consts = build_consts(np.asarray(g0o), np.asarray(g1o), np.asarray(g2o))

    nc = _get_nc(_C)
    in_maps = []
    for k in range(_NCORES):
        tbp, ylp = pack_inputs(Yl[k], Yhr[k], Yhi[k])
        in_maps.append({"ylp": ylp, "tbp": tbp, **consts})
    res = run_bass_kernel_spmd(nc, in_maps, list(range(_NCORES)))
    out = np.stack([unpack_output(res.results[k]["out"]) for k in range(_NCORES)])
    return out.astype(np.float32)


# revision 38
# speedup vs baseline: 1.0521x; 1.0521x over previous
"""Inverse DTCWT (biort bandpass) level-1 reconstruction as a Bass/Tile kernel.

Math: the reference is
    y = (A0 @ Yl + A1 @ lh) @ A0^T + (A0 @ hl) @ A1^T + (A2 @ hh) @ A2^T
where A* are 256x256 banded matrices (1D taps + symmetric padding folded in)
and lh/hl/hh are the c2q quad-interleaves of subband pairs (0,5)/(2,3)/(1,4).

Row r of a c2q image comes from `top` (r even) or `bot` (r odd), each a
128x256 column-interleaved image built with 3 tensor-tensor ops per pair:
    top[:, 0::2] = w1r + w2r ; top[:, 1::2] = w1i + w2i
    bot[:, 0::2] = w1i - w2i ; bot[:, 1::2] = w2r - w1r
The row interleave never materializes: contraction over rows splits into
even/odd with host-precomputed matrices Re = A^T[0::2]/sqrt2, Ro = A^T[1::2]/sqrt2.

Stage A (col filters) runs with the image tiles stationary producing
transposed intermediates Z[c, h] in PSUM; stage B (row filters) consumes Z
slices as stationary against A^T and accumulates all three paths into one
PSUM bank in natural orientation. No transposes anywhere.

Everything runs in fp16: halves the HBM traffic vs fp32 and runs the PE at
1 cycle/row at ANY moving width (fp32r needs >=256-wide moving for full
rate). That unlocks banded windows: A^T is banded (13/19/13 taps), so the
Yl stage-A matmuls and all 12 stage-B matmuls stream only the ~134-137
nonzero columns of each 256-wide A^T chunk. PSUM start=True marks the whole
2KB bank pending-zero and each byte's first writer overwrites, so one start
per bank is enough even with partial windows.

top/bot are linear combos of the input subbands, so the host computes them
during packing (c2q is a bijective repack: same DMA bytes, zero on-device
element-wise work). They are packed parity-stacked (partitions 0-63 = top
half-slab, 64-127 = bot half-slab) so one matmul contracts both parities of
a half-height slab against the stacked [Re; Ro] matrix, whose nonzero
window is only 128+m columns. Banks 0-1 use the windowed form; bank 2
streams the full 256 columns on purpose — the PSUM->SBUF copy engines
(vector: z0+out, scalar: z1+z2, ~2.1us/image between them) pin the steady
period anyway, and a longer stage A keeps them below the saturation point
where copy backpressure quantizes into PE stalls.

The per-image schedule is software-pipelined: stage B of image i-1 issues
after stage A of image i, and the copies drain during the opposite stage so
the PE never waits on one. Inputs stream on the sync DMA queue (consts on
scalar, yl/out on gpsimd), group 0 split per-image; a few dummy matmuls
ramp the PE clock while the first transfers land.

Sharding: pure data parallel, batch dim (8) across 8 cores.
"""
import sys

if "/opt/trn_rl_repo" not in sys.path:
    sys.path.insert(0, "/opt/trn_rl_repo")

import numpy as np

_C, _H = 64, 256  # channels per core, image size
_NCORES = 8
_G = 4  # images (channels) per group


def _band_matrix(h, N):
    """A @ x == colfilter(x, h) with symmetric padding, in float64."""
    h = np.asarray(h, dtype=np.float64)
    L = h.shape[0]
    m = L // 2
    A = np.zeros((N, N), dtype=np.float64)
    for i in range(N):
        for k in range(L):
            s = i + k - m
            if s < 0:
                s = -1 - s
            elif s >= N:
                s = 2 * N - 1 - s
            A[i, s] += h[L - 1 - k]
    return A


def build_consts(g0o, g1o, g2o):
    """Host-side constant tensors handed to every core."""
    A0 = _band_matrix(g0o, _H).T  # stored transposed: [r, h]
    A1 = _band_matrix(g1o, _H).T
    A2 = _band_matrix(g2o, _H).T
    s2 = np.sqrt(2.0)

    def tile2(AT):  # [256, 256] -> [128, 2, 256] with [p, kr, h] = AT[128*kr+p, h]
        return np.ascontiguousarray(
            AT.reshape(2, 128, 256).transpose(1, 0, 2)
        ).astype(np.float16)

    a0t, a1t, a2t = tile2(A0), tile2(A1), tile2(A2)
    # rmats[q, e/o]: per-pair col-filter matrices; pair q uses bands (q, 5-q):
    #   q=0 (lh)   -> col filter A1 ; q=1 (hh) -> A2 ; q=2 (hl) -> A0
    rmats = np.stack(
        [
            np.stack([A1[0::2] / s2, A1[1::2] / s2]),
            np.stack([A2[0::2] / s2, A2[1::2] / s2]),
            np.stack([A0[0::2] / s2, A0[1::2] / s2]),
        ]
    )  # [3, e/o, 128, 256]
    # parity-stacked layout [t, q, h, hout]: partition t<64 holds the even
    # (top) rows 64h+t, t>=64 the odd (bot) rows 64h+t-64. One matmul then
    # contracts a half-height slab of BOTH parities, and the nonzero hout
    # window shrinks to 128+m — windowed matmuls stream ~2x faster than
    # 256-wide ones.
    rmp = np.empty((128, 3, 2, 256), dtype=np.float16)
    rmp[:64] = rmats[:, 0].reshape(3, 2, 64, 256).transpose(2, 0, 1, 3)
    rmp[64:] = rmats[:, 1].reshape(3, 2, 64, 256).transpose(2, 0, 1, 3)
    return {"a0t": a0t, "a1t": a1t, "a2t": a2t, "rmats": rmp}


# half-tap margins of the three band matrices (13/19/13 taps)
_M0, _M1, _M2 = 6, 9, 6


def build_nc(n_images):
    import concourse.bacc as bacc
    import concourse.mybir as mybir
    from concourse.tile import TileContext

    f32 = mybir.dt.float32
    f16 = mybir.dt.float16
    nc = bacc.Bacc(None, target_bir_lowering=False, debug=False)

    n_groups = n_images // _G
    yl_d = nc.declare_dram_parameter(
        "ylp", [n_groups, 128, _G, 2, 256], f16, isOutput=False
    )
    tb_d = nc.declare_dram_parameter(
        "tbp", [n_groups, 128, _G, 3, 2, 128, 2], f16, isOutput=False
    )
    a0t_d = nc.declare_dram_parameter("a0t", [128, 2, 256], f16, isOutput=False)
    a1t_d = nc.declare_dram_parameter("a1t", [128, 2, 256], f16, isOutput=False)
    a2t_d = nc.declare_dram_parameter("a2t", [128, 2, 256], f16, isOutput=False)
    rm_d = nc.declare_dram_parameter("rmats", [128, 3, 2, 256], f16, isOutput=False)
    out_d = nc.declare_dram_parameter(
        "out", [n_groups, 128, _G, 2, 256], f16, isOutput=True
    )
    assert n_groups * _G == n_images

    with TileContext(nc) as tc:
        with (
            tc.tile_pool(name="consts", bufs=1) as cpool,
            tc.tile_pool(name="io", bufs=2) as io_pool,
            tc.tile_pool(name="tb", bufs=2) as tb_pool,
            tc.tile_pool(name="zsb", bufs=2) as z_pool,
            tc.tile_pool(name="ps", bufs=2, space="PSUM") as ps_pool,
        ):
            a0t = cpool.tile([128, 2, 256], f16)
            a1t = cpool.tile([128, 2, 256], f16)
            a2t = cpool.tile([128, 2, 256], f16)
            amats = (a0t, a1t, a2t)
            ms = (_M0, _M1, _M2)
            rm = cpool.tile([128, 3, 2, 256], f16)

            # PE p-state warmup: dummy matmuls on zeroed scratch while the
            # first DMAs land, so real work starts at full clock
            scratch = cpool.tile([128, 512], f16)
            nc.vector.memset(scratch[:], 0.0)
            wp = ps_pool.tile([128, 2, 256], f32, tag="yp")
            for _ in range(8):
                nc.tensor.matmul(
                    wp[:, :, :], scratch[:, 0:128], scratch[:],
                    start=True, stop=True,
                )

            # consts on the scalar DMA queue, data on sync: parallel transfers
            nc.scalar.dma_start(rm[:], rm_d[:])
            nc.scalar.dma_start(a0t[:], a0t_d[:])
            nc.scalar.dma_start(a1t[:], a1t_d[:])
            nc.scalar.dma_start(a2t[:], a2t_d[:])

            # software pipeline state: stage B of image idx-1 runs behind
            # stage A of image idx
            prev = None  # (z_s, out_sb, i, g, idx) of the previous image

            def stage_b(z_s, out_sb, i, g, idx):
                # p outer / r inner: consumes z banks in the order their
                # copies complete (bank 0 first)
                yp = ps_pool.tile([128, 2, 256], f32, tag="yp")
                nmm = 0
                for p in range(3):
                    m = ms[p]
                    for r in range(2):
                        rs = slice(128 * r, 128 * r + 128)
                        for k in range(2):
                            w0, w1 = (0, 128 + m) if k == 0 else (128 - m, 256)
                            nc.tensor.matmul(
                                yp[:, r, w0:w1],
                                z_s[:, p, k, rs],
                                amats[p][:, k, w0:w1],
                                start=(nmm == 0),
                                stop=(nmm == 11),
                            )
                            nmm += 1
                # out copy on vector; DMA issues from the gpsimd queue
                nc.vector.tensor_copy(out_sb[:, i, :, :], yp[:])
                nc.gpsimd.dma_start(out_d[g][:, i], out_sb[:, i])

            for idx in range(n_images):
                g, i = divmod(idx, _G)
                if i == 0:
                    tb = tb_pool.tile(
                        [128, _G, 3, 2, 128, 2], f16, tag="tb", bufs=3
                    )
                    yl = io_pool.tile([128, _G, 2, 256], f16, tag="yl")
                    out_sb = io_pool.tile([128, _G, 2, 256], f16, tag="out_sb")
                    # group 0 split per-image so early images land first
                    if g == 0:
                        nc.sync.dma_start(tb[:, 0:1], tb_d[g][:, 0:1])
                        nc.sync.dma_start(yl[:, 0:1], yl_d[g][:, 0:1])
                        nc.sync.dma_start(tb[:, 1:2], tb_d[g][:, 1:2])
                        nc.sync.dma_start(yl[:, 1:_G], yl_d[g][:, 1:_G])
                        nc.sync.dma_start(tb[:, 2:_G], tb_d[g][:, 2:_G])
                    else:
                        nc.sync.dma_start(tb[:], tb_d[g])
                        nc.gpsimd.dma_start(yl[:], yl_d[g])

                # ---- stage A: Z[c, h] = col-filtered, transposed ----
                # z banks: 0 = y1 path (lh via rm[0] + Yl via a0t, row filter A0)
                #          1 = hl path (pair q=2, col rm[2], row filter A1)
                #          2 = hh path (pair q=1, col rm[1], row filter A2)
                z = ps_pool.tile([128, 3, 2, 256], f32, tag="z")

                # per-q col-filter margins: q=0 (lh) uses A1, q=1 (hh) A2,
                # q=2 (hl) A0
                colm = (_M1, _M2, _M0)

                def subs(bank, q, last_stop, windowed, i=i, z=z):
                    mq = colm[q]
                    for cc in range(2):
                        js = slice(64 * cc, 64 * cc + 64)
                        for h in range(2):
                            if windowed:
                                w0, w1 = (
                                    (0, 128 + mq) if h == 0 else (128 - mq, 256)
                                )
                            else:
                                w0, w1 = 0, 256
                            nc.tensor.matmul(
                                z[:, bank, cc, w0:w1],
                                tb[:, i, q, h, js, :],
                                rm[:, q, h, w0:w1],
                                start=(cc == 0 and h == 0),
                                stop=(last_stop and cc == 1 and h == 1),
                            )

                def bank0(i=i, z=z):
                    subs(0, 0, last_stop=False, windowed=True)
                    for cc in range(2):
                        ws = slice(128 * cc, 128 * cc + 128)
                        for k in range(2):
                            w0, w1 = (
                                (0, 128 + _M0) if k == 0 else (128 - _M0, 256)
                            )
                            nc.tensor.matmul(
                                z[:, 0, cc, w0:w1], yl[:, i, k, ws],
                                a0t[:, k, w0:w1],
                                start=False, stop=(cc == 1 and k == 1),
                            )

                def bank12(i=i, z=z):
                    subs(1, 2, last_stop=True, windowed=True)
                    subs(2, 1, last_stop=True, windowed=False)

                # bank 0 first so its copy (stage B's first dependency)
                # starts earliest; in the first group the yl transfer is
                # still in flight, so run the tb-only banks first instead
                if idx < _G:
                    bank12()
                    bank0()
                else:
                    bank0()
                    bank12()

                # PSUM -> SBUF fp16 in bank-completion order, balanced
                # across vector + scalar (bank 1 split between them)
                z_s = z_pool.tile([128, 3, 2, 256], f16, tag="zs")
                nc.vector.tensor_copy(z_s[:, 0], z[:, 0])
                nc.scalar.copy(z_s[:, 1:3], z[:, 1:3])

                # ---- stage B for the previous image ----
                if prev is not None:
                    stage_b(*prev)
                prev = (z_s, out_sb, i, g, idx)

            stage_b(*prev)
    nc.compile()
    return nc


_NC_CACHE = {}


def _get_nc(n_images):
    if n_images not in _NC_CACHE:
        _NC_CACHE[n_images] = build_nc(n_images)
    return _NC_CACHE[n_images]


def pack_inputs(Yl_k, Yhr_k, Yhi_k):
    """Per-core repack into group-major fp16 layouts with long contiguous rows.

    The c2q column-interleaved top/bot half-images are linear combos of the
    subbands; build them here so the device does matmuls only.
    tbp[g, h, i, q, t/b, w, ri]                       -> 12KB/partition/group
    ylp[g, p, i, k, w] = Yl[4g+i, 128k+p, w]          ->  4KB/partition/group
    """
    ng = _C // _G
    R = Yhr_k.reshape(ng, _G, 6, 128, 128)
    I = Yhi_k.reshape(ng, _G, 6, 128, 128)
    # parity-stacked: partition p<64 holds top rows 64h+p, p>=64 bot rows
    # 64h+p-64, matching the stacked rmats layout
    tbp = np.empty((ng, 128, _G, 3, 2, 128, 2), dtype=np.float16)
    for q in range(3):
        w1r, w2r = R[:, :, q], R[:, :, 5 - q]
        w1i, w2i = I[:, :, q], I[:, :, 5 - q]
        for comp, dst in (
            (w1r + w2r, tbp[:, 0:64, :, q, :, :, 0]),
            (w1i + w2i, tbp[:, 0:64, :, q, :, :, 1]),
            (w1i - w2i, tbp[:, 64:128, :, q, :, :, 0]),
            (w2r - w1r, tbp[:, 64:128, :, q, :, :, 1]),
        ):
            # [ng, G, 128h'', w] -> split h''=64h+p -> [ng, p, G, h, w]
            dst[:] = comp.reshape(ng, _G, 2, 64, 128).transpose(0, 3, 1, 2, 4)
    ylp = np.ascontiguousarray(
        Yl_k.reshape(ng, _G, 2, 128, 256).transpose(0, 3, 1, 2, 4)
    ).astype(np.float16)
    return tbp, ylp


def unpack_output(outp):
    """outp (ng, 128, G, 2, 256): [g, p, i, k, w] = y[Gg+i, 128k+p, w]."""
    return np.ascontiguousarray(
        outp.transpose(0, 2, 3, 1, 4).reshape(outp.shape[0] * _G, 256, 256)
    )


def kernel(Yl, Yhr, Yhi, g0o, g1o, g2o):
    from concourse.bass_utils import run_bass_kernel_spmd

    # banded windows in build_nc assume 13/19/13-tap filters
    assert len(g0o) == 13 and len(g1o) == 19 and len(g2o) == 13

    Yl = np.asarray(Yl, dtype=np.float32)
    Yhr = np.asarray(Yhr, dtype=np.float32)
    Yhi = np.asarray(Yhi, dtype=np.float32)
    consts = build_consts(np.asarray(g0o), np.asarray(g1o), np.asarray(g2o))

    nc = _get_nc(_C)
    in_maps = []
    for k in range(_NCORES):
        tbp, ylp = pack_inputs(Yl[k], Yhr[k], Yhi[k])
        in_maps.append({"ylp": ylp, "tbp": tbp, **consts})
    res = run_bass_kernel_spmd(nc, in_maps, list(range(_NCORES)))
    out = np.stack([unpack_output(res.results[k]["out"]) for k in range(_NCORES)])
    return out.astype(np.float32)
